# revision 28
# baseline (speedup 1.0000x reference)
"""Binarized conv block (BinBlock) Trainium2 Bass kernel.

Reference computation (per image):
    xb    = sign(x)                                  # +/-1
    alpha = mean|W| over (I,kh,kw)                   # [O]
    wb    = alpha * sign(W)
    xp    = pad(xb, 1, value=-1)
    out   = conv2d(xp, wb) + bias
    out   = out*gBN + (beta - mean*gBN),  gBN = gamma/sqrt(var+eps)
    out   = out + x

Kernel algebra: let s = alpha*gBN, b2 = bias*gBN + beta - mean*gBN.
    out = s * conv2d(pad(sign(x),-1), sign(W)) + b2 + x
We binarize to {+0.5,-0.5} (exact in bf16; pad = -0.5), so the integer conv
appears scaled by 0.5, and inject the residual into the same PSUM
accumulation through a diag(1/(2s)) matmul:
    psum = 0.5*conv_int + x/(2s)
    out  = psum*(2s) + b2        (single scalar-engine activation)

Sharding: batch 32 -> 4 images per core on 8 cores. Per core, images are
processed in pairs: image parity selects the SBUF partition half (input
row-group of the PE array); block parity selects the PSUM partition half
(output column-group). That drives all four 64x64 PE array tiles
concurrently with K=M=64 matmuls.
"""

import numpy as np
import ml_dtypes

import concourse.bass as bass
import concourse.bacc as bacc
import concourse.tile as tile
import concourse.mybir as mybir
from concourse import bass_utils

F32 = mybir.dt.float32
F32R = mybir.dt.float32r
BF16 = mybir.dt.bfloat16

B, C, H, W = 32, 64, 112, 112
NCORES = 8
BSH = B // NCORES          # images per core
HWF = H * W                # 12544
HP = H + 2                 # 114 padded
PADN = HP * HP             # 12996
ROWS_PER_BLK = 4
NBLK = H // ROWS_PER_BLK   # 28
NB = ROWS_PER_BLK * W      # 448 (fits one PSUM bank: 512 fp32)
HALF_BLKS = NBLK // 2      # 14: blocks per psum-half stream
GBLK = 4                   # blocks per output stage group
BN_EPS = 1e-5

ACT_COPY = mybir.ActivationFunctionType.Identity
OP_GE = mybir.AluOpType.is_ge
OP_SUB = mybir.AluOpType.subtract
OP_MULT = mybir.AluOpType.mult
OP_ADD = mybir.AluOpType.add


def build_kernel_body(tc, out_d, x_d, ws_d, wd_d, sb_d):
    nc = tc.nc
    with (
        tc.tile_pool(name="const", bufs=1) as constp,
        tc.tile_pool(name="xraw", bufs=2) as xrawp,
        tc.tile_pool(name="sign", bufs=2) as signp,
        tc.tile_pool(name="stage", bufs=6) as stagep,
        tc.tile_pool(name="psum", bufs=8, space="PSUM") as psump,
    ):
        ws_t = constp.tile([128, 9 * C], BF16)   # sign(W)^T per position
        nc.sync.dma_start(ws_t[:], ws_d[:])
        wd_t = constp.tile([128, C], F32)        # diag(1/(2s))
        nc.sync.dma_start(wd_t[:], wd_d[:])
        sb_t = constp.tile([128, 2], F32)        # col0: 2s, col1: b2
        nc.sync.dma_start(sb_t[:], sb_d[:])
        sc_t = sb_t[:, 0:1]
        bi_t = sb_t[:, 1:2]

        # PE warm-up: ~55 throwaway matmuls on the weight tile keep the PE
        # busy (and the HAM clock-gate at 8/8) while the first input chunks
        # stream in; the first real matmul then issues at full clock.
        wps = psump.tile([128, 512], F32, name="warm_ps", tag="ps")
        for wi in range(55):
            nc.tensor.matmul(
                wps[0:64, :], ws_t[0:64, 0:64], ws_t[0:64, 0:512],
                start=True, stop=True,
            )

        CHUNKS = ((0, 24), (24, 56), (56, 88), (88, H))

        def pro_alloc(p):
            xr = xrawp.tile([128, HWF], F32, name=f"xr_{p}", tag="xr")
            xr3 = xr[:].rearrange("p (h w) -> p h w", w=W)
            sg = signp.tile([128, PADN], BF16, name=f"sg_{p}", tag="sg")
            sg3 = sg[:].rearrange("p (h w) -> p h w", w=HP)
            # -0.5 padding border (top/bottom rows, left/right columns)
            nc.vector.memset(sg3[:, 0, :], -0.5)
            nc.vector.memset(sg3[:, HP - 1, :], -0.5)
            nc.vector.memset(sg3[:, 1 : HP - 1, 0], -0.5)
            nc.vector.memset(sg3[:, 1 : HP - 1, HP - 1], -0.5)
            return xr, xr3, sg3

        def pro_chunk(p, xr3, sg3, ci):
            # load + binarize one row chunk: (x >= 0) - 0.5  ->  {+0.5, -0.5}
            ra, rb = CHUNKS[ci]
            nc.sync.dma_start(
                xr3[:, ra:rb, :],
                x_d[2 * p : 2 * p + 2, :, ra:rb, :].rearrange(
                    "b c h w -> (b c) (h w)"
                ),
            )
            nc.vector.tensor_scalar(
                sg3[:, 1 + ra : 1 + rb, 1 : HP - 1],
                xr3[:, ra:rb, :],
                0.0,
                0.5,
                OP_GE,
                OP_SUB,
            )

        pro = {0: pro_alloc(0)}
        for ci in range(len(CHUNKS)):
            pro_chunk(0, pro[0][1], pro[0][2], ci)
        for p in range(BSH // 2):  # image pairs; image 2p -> partitions 0:64, 2p+1 -> 64:128
            if p + 1 < BSH // 2:
                pro[p + 1] = pro_alloc(p + 1)
                for ci in range(len(CHUNKS)):
                    pro_chunk(p + 1, pro[p + 1][1], pro[p + 1][2], ci)
            xr, _, sg3 = pro.pop(p)

            # Stream q = (image-half ih, psum-half hf). Each 4-block group m
            # is split hf=0 -> blocks 4m..4m+1, hf=1 -> 4m+2..4m+3, so step 0
            # only needs the first input rows and each out-DMA still covers
            # 16 contiguous DRAM rows.
            stages = [None, None]
            for m in range(NBLK // 4):  # 7 groups of 4 blocks
                sj = 2
                for j in range(sj):
                    psums = [
                        psump.tile(
                            [128, NB], F32, name=f"ps_p{p}m{m}j{j}q{q}", tag="ps"
                        )
                        for q in range(4)
                    ]
                    # 9 conv positions, round-robin over the 4 array tiles
                    for pos in range(9):
                        dh, dw = divmod(pos, 3)
                        for q in range(4):
                            ih, hf = divmod(q, 2)
                            blk = 4 * m + sj * hf + j
                            r0 = 4 * blk + dh
                            nc.tensor.matmul(
                                psums[q][64 * hf : 64 * hf + 64, :],
                                ws_t[64 * ih : 64 * ih + 64, 64 * pos : 64 * pos + 64],
                                sg3[64 * ih : 64 * ih + 64, r0 : r0 + 4, dw : dw + W],
                                start=(pos == 0),
                                stop=False,
                            )
                    # residual: psum += diag(1/(2s)) @ x_block   (fp32)
                    for q in range(4):
                        ih, hf = divmod(q, 2)
                        blk = 4 * m + sj * hf + j
                        nc.tensor.matmul(
                            psums[q][64 * hf : 64 * hf + 64, :],
                            wd_t[64 * ih : 64 * ih + 64, :],
                            xr[64 * ih : 64 * ih + 64, blk * NB : (blk + 1) * NB],
                            start=False,
                            stop=True,
                        )
                    # epilogue: out = psum*(2s) + b2; hf=0 on ScalarE, hf=1 on
                    # VectorE so the two drains run on separate engines
                    for ih in range(2):
                        if j == 0:
                            stages[ih] = stagep.tile(
                                [128, sj * NB], F32, name=f"st_p{p}m{m}i{ih}", tag="st"
                            )
                        st = stages[ih]
                        for hf in range(2):
                            q = 2 * ih + hf
                            sl = slice(64 * hf, 64 * hf + 64)
                            if hf == 0:
                                nc.scalar.activation(
                                    st[sl, j * NB : (j + 1) * NB],
                                    psums[q][sl, :],
                                    ACT_COPY,
                                    bias=sb_t[sl, 1:2],
                                    scale=sb_t[sl, 0:1],
                                )
                            else:
                                nc.vector.tensor_scalar(
                                    st[sl, j * NB : (j + 1) * NB],
                                    psums[q][sl, :],
                                    sb_t[sl, 0:1],
                                    sb_t[sl, 1:2],
                                    OP_MULT,
                                    OP_ADD,
                                )
                        if j == sj - 1:
                            n = 2 * p + ih
                            dst = out_d[n, :, 16 * m : 16 * m + 8 * sj, :].rearrange(
                                "c (b rr) w -> b c (rr w)", b=2
                            )
                            nc.gpsimd.dma_start(dst, st[:])


def build_nc():
    nc = bacc.Bacc(trn_type="TRN2", debug=False, num_devices=NCORES)
    x_d = nc.dram_tensor("x", [BSH, C, H, W], F32, kind="ExternalInput")
    ws_d = nc.dram_tensor("wsign", [128, 9 * C], BF16, kind="ExternalInput")
    wd_d = nc.dram_tensor("wdiag", [128, C], F32, kind="ExternalInput")
    sb_d = nc.dram_tensor("scalebias", [128, 2], F32, kind="ExternalInput")
    out_d = nc.dram_tensor("out", [BSH, C, H, W], F32, kind="ExternalOutput")
    with tile.TileContext(nc) as tc:
        build_kernel_body(tc, out_d, x_d, ws_d, wd_d, sb_d)
    nc.compile()
    return nc


def prep_consts(weight, bias, gamma, beta, run_mean, run_var):
    """Host-side constant prep (numpy, fp64 for the folding math)."""
    w = np.asarray(weight, np.float64)
    alpha = np.mean(np.abs(w), axis=(1, 2, 3))            # [O]
    g = np.asarray(gamma, np.float64) / np.sqrt(np.asarray(run_var, np.float64) + BN_EPS)
    s = alpha * g                                          # [O]
    b2 = np.asarray(bias, np.float64) * g + np.asarray(beta, np.float64) - np.asarray(
        run_mean, np.float64
    ) * g

    wsign = np.sign(w)                                     # [O,I,3,3]
    # lhsT layout [I(dup to 128), pos, O]
    ws = wsign.transpose(1, 2, 3, 0).reshape(C, 9, C).transpose(0, 1, 2)
    ws = ws.reshape(C, 9 * C)
    ws128 = np.concatenate([ws, ws], axis=0).astype(ml_dtypes.bfloat16)

    wd = np.zeros((C, C), np.float64)
    np.fill_diagonal(wd, 1.0 / (2.0 * s))
    wd128 = np.concatenate([wd, wd], axis=0).astype(np.float32)

    sc = np.concatenate([2.0 * s, 2.0 * s]).astype(np.float32)
    bi = np.concatenate([b2, b2]).astype(np.float32)
    sb128 = np.stack([sc, bi], axis=1)  # [128, 2]
    return ws128, wd128, sb128


_CACHE = {}


def kernel(x, weight, bias, gamma, beta, run_mean, run_var, _trace=False, _trace_kwargs=None):
    x = np.ascontiguousarray(np.asarray(x, np.float32))
    ws128, wd128, sb128 = prep_consts(weight, bias, gamma, beta, run_mean, run_var)

    if "nc" not in _CACHE:
        _CACHE["nc"] = build_nc()
    nc = _CACHE["nc"]

    in_maps = []
    for i in range(NCORES):
        in_maps.append(
            dict(
                x=x[BSH * i : BSH * (i + 1)],
                wsign=ws128,
                wdiag=wd128,
                scalebias=sb128,
            )
        )
    res = bass_utils.run_bass_kernel_spmd(
        nc,
        in_maps,
        core_ids=list(range(NCORES)),
        trace=_trace,
        **(_trace_kwargs or {}),
    )
    out = np.concatenate([res.results[i]["out"] for i in range(NCORES)], axis=0)
    if _trace:
        kernel.last_results = res
    return out


# revision 29
# speedup vs baseline: 1.0740x; 1.0740x over previous
"""Binarized conv block (BinBlock) Trainium2 Bass kernel.

Reference computation (per image):
    xb    = sign(x)                                  # +/-1
    alpha = mean|W| over (I,kh,kw)                   # [O]
    wb    = alpha * sign(W)
    xp    = pad(xb, 1, value=-1)
    out   = conv2d(xp, wb) + bias
    out   = out*gBN + (beta - mean*gBN),  gBN = gamma/sqrt(var+eps)
    out   = out + x

Kernel algebra: let s = alpha*gBN, b2 = bias*gBN + beta - mean*gBN.
    out = s * conv2d(pad(sign(x),-1), sign(W)) + b2 + x
We binarize to {+0.5,-0.5} (exact in bf16; pad = -0.5), so the integer conv
appears scaled by 0.5, and inject the residual into the same PSUM
accumulation through a diag(1/(2s)) matmul:
    psum = 0.5*conv_int + x/(2s)
    out  = psum*(2s) + b2        (single scalar-engine activation)

Sharding: batch 32 -> 4 images per core on 8 cores. Per core, images are
processed in pairs: image parity selects the SBUF partition half (input
row-group of the PE array); block parity selects the PSUM partition half
(output column-group). That drives all four 64x64 PE array tiles
concurrently with K=M=64 matmuls.
"""

import numpy as np
import ml_dtypes

import concourse.bass as bass
import concourse.bacc as bacc
import concourse.tile as tile
import concourse.mybir as mybir
from concourse import bass_utils

F32 = mybir.dt.float32
F32R = mybir.dt.float32r
BF16 = mybir.dt.bfloat16

B, C, H, W = 32, 64, 112, 112
NCORES = 8
BSH = B // NCORES          # images per core
HWF = H * W                # 12544
HP = H + 2                 # 114 padded
PADN = HP * HP             # 12996
ROWS_PER_BLK = 4
NBLK = H // ROWS_PER_BLK   # 28
NB = ROWS_PER_BLK * W      # 448 (fits one PSUM bank: 512 fp32)
HALF_BLKS = NBLK // 2      # 14: blocks per psum-half stream
GBLK = 4                   # blocks per output stage group
BN_EPS = 1e-5

ACT_COPY = mybir.ActivationFunctionType.Identity
OP_GE = mybir.AluOpType.is_ge
OP_SUB = mybir.AluOpType.subtract
OP_MULT = mybir.AluOpType.mult
OP_ADD = mybir.AluOpType.add


def build_kernel_body(tc, out_d, x_d, ws_d, wd_d, sb_d):
    nc = tc.nc
    with (
        tc.tile_pool(name="const", bufs=1) as constp,
        tc.tile_pool(name="xraw", bufs=2) as xrawp,
        tc.tile_pool(name="sign", bufs=2) as signp,
        tc.tile_pool(name="stage", bufs=8) as stagep,
        tc.tile_pool(name="psum", bufs=8, space="PSUM") as psump,
    ):
        ws_t = constp.tile([128, 9 * C], BF16)   # sign(W)^T per position
        nc.sync.dma_start(ws_t[:], ws_d[:])
        wd_t = constp.tile([128, C], F32)        # diag(1/(2s))
        nc.sync.dma_start(wd_t[:], wd_d[:])
        sb_t = constp.tile([128, 2], F32)        # col0: 2s, col1: b2
        nc.sync.dma_start(sb_t[:], sb_d[:])
        sc_t = sb_t[:, 0:1]
        bi_t = sb_t[:, 1:2]

        CHUNKS = ((0, 24), (24, 56), (56, 88), (88, H))

        def pro_alloc(p):
            xr = xrawp.tile([128, HWF], F32, name=f"xr_{p}", tag="xr")
            xr3 = xr[:].rearrange("p (h w) -> p h w", w=W)
            sg = signp.tile([128, PADN], BF16, name=f"sg_{p}", tag="sg")
            sg3 = sg[:].rearrange("p (h w) -> p h w", w=HP)
            # -0.5 padding border (top/bottom rows, left/right columns)
            nc.vector.memset(sg3[:, 0, :], -0.5)
            nc.vector.memset(sg3[:, HP - 1, :], -0.5)
            nc.vector.memset(sg3[:, 1 : HP - 1, 0], -0.5)
            nc.vector.memset(sg3[:, 1 : HP - 1, HP - 1], -0.5)
            return xr, xr3, sg3

        def pro_chunk(p, xr3, sg3, ci):
            # load + binarize one row chunk: (x >= 0) - 0.5  ->  {+0.5, -0.5}
            ra, rb = CHUNKS[ci]
            nc.sync.dma_start(
                xr3[:, ra:rb, :],
                x_d[2 * p : 2 * p + 2, :, ra:rb, :].rearrange(
                    "b c h w -> (b c) (h w)"
                ),
            )
            nc.vector.tensor_scalar(
                sg3[:, 1 + ra : 1 + rb, 1 : HP - 1],
                xr3[:, ra:rb, :],
                0.0,
                0.5,
                OP_GE,
                OP_SUB,
            )

        pro = {0: pro_alloc(0)}
        for ci in range(len(CHUNKS)):
            pro_chunk(0, pro[0][1], pro[0][2], ci)
        for p in range(BSH // 2):  # image pairs; image 2p -> partitions 0:64, 2p+1 -> 64:128
            if p + 1 < BSH // 2:
                pro[p + 1] = pro_alloc(p + 1)
                for ci in range(len(CHUNKS)):
                    pro_chunk(p + 1, pro[p + 1][1], pro[p + 1][2], ci)
            xr, _, sg3 = pro.pop(p)

            # Stream q = (image-half ih, psum-half hf). Each 4-block group m
            # is split hf=0 -> blocks 4m..4m+1, hf=1 -> 4m+2..4m+3, so step 0
            # only needs the first input rows and each out-DMA still covers
            # 16 contiguous DRAM rows.
            stages = [None, None]
            for m in range(NBLK // 4):  # 7 groups of 4 blocks
                sj = 2
                for j in range(sj):
                    psums = [
                        psump.tile(
                            [128, NB], F32, name=f"ps_p{p}m{m}j{j}q{q}", tag="ps"
                        )
                        for q in range(4)
                    ]
                    # 9 conv positions, round-robin over the 4 array tiles
                    for pos in range(9):
                        dh, dw = divmod(pos, 3)
                        for q in range(4):
                            ih, hf = divmod(q, 2)
                            blk = 4 * m + sj * hf + j
                            r0 = 4 * blk + dh
                            nc.tensor.matmul(
                                psums[q][64 * hf : 64 * hf + 64, :],
                                ws_t[64 * ih : 64 * ih + 64, 64 * pos : 64 * pos + 64],
                                sg3[64 * ih : 64 * ih + 64, r0 : r0 + 4, dw : dw + W],
                                start=(pos == 0),
                                stop=False,
                            )
                    # residual: psum += diag(1/(2s)) @ x_block   (fp32)
                    for q in range(4):
                        ih, hf = divmod(q, 2)
                        blk = 4 * m + sj * hf + j
                        nc.tensor.matmul(
                            psums[q][64 * hf : 64 * hf + 64, :],
                            wd_t[64 * ih : 64 * ih + 64, :],
                            xr[64 * ih : 64 * ih + 64, blk * NB : (blk + 1) * NB],
                            start=False,
                            stop=True,
                        )
                    # epilogue: out = psum*(2s) + b2; hf=0 on ScalarE, hf=1 on
                    # VectorE so the two drains run on separate engines
                    for ih in range(2):
                        if j == 0:
                            stages[ih] = stagep.tile(
                                [128, sj * NB], F32, name=f"st_p{p}m{m}i{ih}", tag="st"
                            )
                        st = stages[ih]
                        for hf in range(2):
                            q = 2 * ih + hf
                            sl = slice(64 * hf, 64 * hf + 64)
                            if hf == 0:
                                nc.scalar.activation(
                                    st[sl, j * NB : (j + 1) * NB],
                                    psums[q][sl, :],
                                    ACT_COPY,
                                    bias=sb_t[sl, 1:2],
                                    scale=sb_t[sl, 0:1],
                                )
                            else:
                                nc.vector.tensor_scalar(
                                    st[sl, j * NB : (j + 1) * NB],
                                    psums[q][sl, :],
                                    sb_t[sl, 0:1],
                                    sb_t[sl, 1:2],
                                    OP_MULT,
                                    OP_ADD,
                                )
                        if j == sj - 1:
                            n = 2 * p + ih
                            dst = out_d[n, :, 16 * m : 16 * m + 8 * sj, :].rearrange(
                                "c (b rr) w -> b c (rr w)", b=2
                            )
                            nc.gpsimd.dma_start(dst, st[:])


def build_nc():
    nc = bacc.Bacc(trn_type="TRN2", debug=False, num_devices=NCORES)
    x_d = nc.dram_tensor("x", [BSH, C, H, W], F32, kind="ExternalInput")
    ws_d = nc.dram_tensor("wsign", [128, 9 * C], BF16, kind="ExternalInput")
    wd_d = nc.dram_tensor("wdiag", [128, C], F32, kind="ExternalInput")
    sb_d = nc.dram_tensor("scalebias", [128, 2], F32, kind="ExternalInput")
    out_d = nc.dram_tensor("out", [BSH, C, H, W], F32, kind="ExternalOutput")
    with tile.TileContext(nc) as tc:
        build_kernel_body(tc, out_d, x_d, ws_d, wd_d, sb_d)
    nc.compile()
    return nc


def prep_consts(weight, bias, gamma, beta, run_mean, run_var):
    """Host-side constant prep (numpy, fp64 for the folding math)."""
    w = np.asarray(weight, np.float64)
    alpha = np.mean(np.abs(w), axis=(1, 2, 3))            # [O]
    g = np.asarray(gamma, np.float64) / np.sqrt(np.asarray(run_var, np.float64) + BN_EPS)
    s = alpha * g                                          # [O]
    b2 = np.asarray(bias, np.float64) * g + np.asarray(beta, np.float64) - np.asarray(
        run_mean, np.float64
    ) * g

    wsign = np.sign(w)                                     # [O,I,3,3]
    # lhsT layout [I(dup to 128), pos, O]
    ws = wsign.transpose(1, 2, 3, 0).reshape(C, 9, C).transpose(0, 1, 2)
    ws = ws.reshape(C, 9 * C)
    ws128 = np.concatenate([ws, ws], axis=0).astype(ml_dtypes.bfloat16)

    wd = np.zeros((C, C), np.float64)
    np.fill_diagonal(wd, 1.0 / (2.0 * s))
    wd128 = np.concatenate([wd, wd], axis=0).astype(np.float32)

    sc = np.concatenate([2.0 * s, 2.0 * s]).astype(np.float32)
    bi = np.concatenate([b2, b2]).astype(np.float32)
    sb128 = np.stack([sc, bi], axis=1)  # [128, 2]
    return ws128, wd128, sb128


_CACHE = {}


def kernel(x, weight, bias, gamma, beta, run_mean, run_var, _trace=False, _trace_kwargs=None):
    x = np.ascontiguousarray(np.asarray(x, np.float32))
    ws128, wd128, sb128 = prep_consts(weight, bias, gamma, beta, run_mean, run_var)

    if "nc" not in _CACHE:
        _CACHE["nc"] = build_nc()
    nc = _CACHE["nc"]

    in_maps = []
    for i in range(NCORES):
        in_maps.append(
            dict(
                x=x[BSH * i : BSH * (i + 1)],
                wsign=ws128,
                wdiag=wd128,
                scalebias=sb128,
            )
        )
    res = bass_utils.run_bass_kernel_spmd(
        nc,
        in_maps,
        core_ids=list(range(NCORES)),
        trace=_trace,
        **(_trace_kwargs or {}),
    )
    out = np.concatenate([res.results[i]["out"] for i in range(NCORES)], axis=0)
    if _trace:
        kernel.last_results = res
    return out


# revision 30
# speedup vs baseline: 1.1298x; 1.0520x over previous
"""Binarized conv block (BinBlock) Trainium2 Bass kernel.

Reference computation (per image):
    xb    = sign(x)                                  # +/-1
    alpha = mean|W| over (I,kh,kw)                   # [O]
    wb    = alpha * sign(W)
    xp    = pad(xb, 1, value=-1)
    out   = conv2d(xp, wb) + bias
    out   = out*gBN + (beta - mean*gBN),  gBN = gamma/sqrt(var+eps)
    out   = out + x

Kernel algebra: let s = alpha*gBN, b2 = bias*gBN + beta - mean*gBN.
    out = s * conv2d(pad(sign(x),-1), sign(W)) + b2 + x
We binarize to {+0.5,-0.5} (exact in bf16; pad = -0.5), so the integer conv
appears scaled by 0.5, and inject the residual into the same PSUM
accumulation through a diag(1/(2s)) matmul:
    psum = 0.5*conv_int + x/(2s)
    out  = psum*(2s) + b2        (single scalar-engine activation)

Sharding: batch 32 -> 4 images per core on 8 cores. Per core, images are
processed in pairs: image parity selects the SBUF partition half (input
row-group of the PE array); block parity selects the PSUM partition half
(output column-group). That drives all four 64x64 PE array tiles
concurrently with K=M=64 matmuls.
"""

import numpy as np
import ml_dtypes

import concourse.bass as bass
import concourse.bacc as bacc
import concourse.tile as tile
import concourse.mybir as mybir
from concourse import bass_utils

F32 = mybir.dt.float32
F32R = mybir.dt.float32r
BF16 = mybir.dt.bfloat16

B, C, H, W = 32, 64, 112, 112
NCORES = 8
BSH = B // NCORES          # images per core
HWF = H * W                # 12544
HP = H + 2                 # 114 padded
PADN = HP * HP             # 12996
ROWS_PER_BLK = 4
NBLK = H // ROWS_PER_BLK   # 28
NB = ROWS_PER_BLK * W      # 448 (fits one PSUM bank: 512 fp32)
HALF_BLKS = NBLK // 2      # 14: blocks per psum-half stream
GBLK = 4                   # blocks per output stage group
BN_EPS = 1e-5

ACT_COPY = mybir.ActivationFunctionType.Identity
OP_GE = mybir.AluOpType.is_ge
OP_SUB = mybir.AluOpType.subtract
OP_MULT = mybir.AluOpType.mult
OP_ADD = mybir.AluOpType.add


def build_kernel_body(tc, out_d, x_d, ws_d, wd_d, sb_d):
    nc = tc.nc
    with (
        tc.tile_pool(name="const", bufs=1) as constp,
        tc.tile_pool(name="xraw", bufs=2) as xrawp,
        tc.tile_pool(name="sign", bufs=2) as signp,
        tc.tile_pool(name="stage", bufs=6) as stagep,
        tc.tile_pool(name="psum", bufs=8, space="PSUM") as psump,
    ):
        ws_t = constp.tile([128, 9 * C], BF16)   # sign(W)^T per position
        nc.sync.dma_start(ws_t[:], ws_d[:])
        wd_t = constp.tile([128, C], F32)        # diag(1/(2s))
        nc.sync.dma_start(wd_t[:], wd_d[:])
        sb_t = constp.tile([128, 2], F32)        # col0: 2s, col1: b2
        nc.sync.dma_start(sb_t[:], sb_d[:])
        sc_t = sb_t[:, 0:1]
        bi_t = sb_t[:, 1:2]

        CHUNKS = ((0, 24), (24, 56), (56, 88), (88, H))

        def pro_alloc(p):
            xr = xrawp.tile([128, HWF], F32, name=f"xr_{p}", tag="xr")
            xr3 = xr[:].rearrange("p (h w) -> p h w", w=W)
            sg = signp.tile([128, PADN], BF16, name=f"sg_{p}", tag="sg")
            sg3 = sg[:].rearrange("p (h w) -> p h w", w=HP)
            # -0.5 padding border (top/bottom rows, left/right columns)
            nc.vector.memset(sg3[:, 0, :], -0.5)
            nc.vector.memset(sg3[:, HP - 1, :], -0.5)
            nc.vector.memset(sg3[:, 1 : HP - 1, 0], -0.5)
            nc.vector.memset(sg3[:, 1 : HP - 1, HP - 1], -0.5)
            return xr, xr3, sg3

        def pro_chunk(p, xr3, sg3, ci):
            # load + binarize one row chunk: (x >= 0) - 0.5  ->  {+0.5, -0.5}
            ra, rb = CHUNKS[ci]
            nc.sync.dma_start(
                xr3[:, ra:rb, :],
                x_d[2 * p : 2 * p + 2, :, ra:rb, :].rearrange(
                    "b c h w -> (b c) (h w)"
                ),
            )
            nc.vector.tensor_scalar(
                sg3[:, 1 + ra : 1 + rb, 1 : HP - 1],
                xr3[:, ra:rb, :],
                0.0,
                0.5,
                OP_GE,
                OP_SUB,
            )

        pro = {0: pro_alloc(0)}
        for ci in range(len(CHUNKS)):
            pro_chunk(0, pro[0][1], pro[0][2], ci)
        for p in range(BSH // 2):  # image pairs; image 2p -> partitions 0:64, 2p+1 -> 64:128
            if p + 1 < BSH // 2:
                pro[p + 1] = pro_alloc(p + 1)
                for ci in range(len(CHUNKS)):
                    pro_chunk(p + 1, pro[p + 1][1], pro[p + 1][2], ci)
            xr, _, sg3 = pro.pop(p)

            # Stream q = (image-half ih, psum-half hf). Each 4-block group m
            # is split hf=0 -> blocks 4m..4m+1, hf=1 -> 4m+2..4m+3, so step 0
            # only needs the first input rows and each out-DMA still covers
            # 16 contiguous DRAM rows.
            stages = [None, None]
            for m in range(NBLK // 4):  # 7 groups of 4 blocks
                sj = 2
                for j in range(sj):
                    psums = [
                        psump.tile(
                            [128, NB], F32, name=f"ps_p{p}m{m}j{j}q{q}", tag="ps"
                        )
                        for q in range(4)
                    ]
                    # 9 conv positions, round-robin over the 4 array tiles
                    for pos in range(9):
                        dh, dw = divmod(pos, 3)
                        for q in range(4):
                            ih, hf = divmod(q, 2)
                            blk = 4 * m + sj * hf + j
                            r0 = 4 * blk + dh
                            nc.tensor.matmul(
                                psums[q][64 * hf : 64 * hf + 64, :],
                                ws_t[64 * ih : 64 * ih + 64, 64 * pos : 64 * pos + 64],
                                sg3[64 * ih : 64 * ih + 64, r0 : r0 + 4, dw : dw + W],
                                start=(pos == 0),
                                stop=False,
                            )
                    # residual: psum += diag(1/(2s)) @ x_block   (fp32)
                    for q in range(4):
                        ih, hf = divmod(q, 2)
                        blk = 4 * m + sj * hf + j
                        nc.tensor.matmul(
                            psums[q][64 * hf : 64 * hf + 64, :],
                            wd_t[64 * ih : 64 * ih + 64, :],
                            xr[64 * ih : 64 * ih + 64, blk * NB : (blk + 1) * NB],
                            start=False,
                            stop=True,
                        )
                    # epilogue: out = psum*(2s) + b2; hf=0 on ScalarE, hf=1 on
                    # VectorE so the two drains run on separate engines
                    for ih in range(2):
                        if j == 0:
                            stages[ih] = stagep.tile(
                                [128, sj * NB], F32, name=f"st_p{p}m{m}i{ih}", tag="st"
                            )
                        st = stages[ih]
                        for hf in range(2):
                            q = 2 * ih + hf
                            sl = slice(64 * hf, 64 * hf + 64)
                            if hf == 0:
                                nc.scalar.activation(
                                    st[sl, j * NB : (j + 1) * NB],
                                    psums[q][sl, :],
                                    ACT_COPY,
                                    bias=sb_t[sl, 1:2],
                                    scale=sb_t[sl, 0:1],
                                )
                            else:
                                nc.vector.tensor_scalar(
                                    st[sl, j * NB : (j + 1) * NB],
                                    psums[q][sl, :],
                                    sb_t[sl, 0:1],
                                    sb_t[sl, 1:2],
                                    OP_MULT,
                                    OP_ADD,
                                )
                        if j == sj - 1:
                            n = 2 * p + ih
                            dst = out_d[n, :, 16 * m : 16 * m + 8 * sj, :].rearrange(
                                "c (b rr) w -> b c (rr w)", b=2
                            )
                            nc.gpsimd.dma_start(dst, st[:])


def build_nc():
    nc = bacc.Bacc(trn_type="TRN2", debug=False, num_devices=NCORES)
    x_d = nc.dram_tensor("x", [BSH, C, H, W], F32, kind="ExternalInput")
    ws_d = nc.dram_tensor("wsign", [128, 9 * C], BF16, kind="ExternalInput")
    wd_d = nc.dram_tensor("wdiag", [128, C], F32, kind="ExternalInput")
    sb_d = nc.dram_tensor("scalebias", [128, 2], F32, kind="ExternalInput")
    out_d = nc.dram_tensor("out", [BSH, C, H, W], F32, kind="ExternalOutput")
    with tile.TileContext(nc) as tc:
        build_kernel_body(tc, out_d, x_d, ws_d, wd_d, sb_d)
    nc.compile()
    return nc


def prep_consts(weight, bias, gamma, beta, run_mean, run_var):
    """Host-side constant prep (numpy, fp64 for the folding math)."""
    w = np.asarray(weight, np.float64)
    alpha = np.mean(np.abs(w), axis=(1, 2, 3))            # [O]
    g = np.asarray(gamma, np.float64) / np.sqrt(np.asarray(run_var, np.float64) + BN_EPS)
    s = alpha * g                                          # [O]
    b2 = np.asarray(bias, np.float64) * g + np.asarray(beta, np.float64) - np.asarray(
        run_mean, np.float64
    ) * g

    wsign = np.sign(w)                                     # [O,I,3,3]
    # lhsT layout [I(dup to 128), pos, O]
    ws = wsign.transpose(1, 2, 3, 0).reshape(C, 9, C).transpose(0, 1, 2)
    ws = ws.reshape(C, 9 * C)
    ws128 = np.concatenate([ws, ws], axis=0).astype(ml_dtypes.bfloat16)

    wd = np.zeros((C, C), np.float64)
    np.fill_diagonal(wd, 1.0 / (2.0 * s))
    wd128 = np.concatenate([wd, wd], axis=0).astype(np.float32)

    sc = np.concatenate([2.0 * s, 2.0 * s]).astype(np.float32)
    bi = np.concatenate([b2, b2]).astype(np.float32)
    sb128 = np.stack([sc, bi], axis=1)  # [128, 2]
    return ws128, wd128, sb128


_CACHE = {}


def kernel(x, weight, bias, gamma, beta, run_mean, run_var, _trace=False, _trace_kwargs=None):
    x = np.ascontiguousarray(np.asarray(x, np.float32))
    ws128, wd128, sb128 = prep_consts(weight, bias, gamma, beta, run_mean, run_var)

    if "nc" not in _CACHE:
        _CACHE["nc"] = build_nc()
    nc = _CACHE["nc"]

    in_maps = []
    for i in range(NCORES):
        in_maps.append(
            dict(
                x=x[BSH * i : BSH * (i + 1)],
                wsign=ws128,
                wdiag=wd128,
                scalebias=sb128,
            )
        )
    res = bass_utils.run_bass_kernel_spmd(
        nc,
        in_maps,
        core_ids=list(range(NCORES)),
        trace=_trace,
        **(_trace_kwargs or {}),
    )
    out = np.concatenate([res.results[i]["out"] for i in range(NCORES)], axis=0)
    if _trace:
        kernel.last_results = res
    return out
